# revision 2
# baseline (speedup 1.0000x reference)
# GCN layer kernel for Trainium2: out[b] = relu((a[b] @ x[b]) @ W) * mask[b]
#
# Sharding: data-parallel over the batch (graph) dim. B=8 graphs, 8 cores,
# one graph per core; W replicated. Inputs are the FULL tensors; shards are
# sliced host-side and the per-core outputs stacked back together.
#
# Per-core dataflow (a: [2048,2048], x: [2048,512], W: [512,512]):
#   - a must be contracted over its column index; TensorE contracts over the
#     partition (row) index of both operands, so a is transposed on-chip via
#     PE transpose (fp32 has no DMA-transpose path), 128x128 tiles.
#   - t^T[f,n] = sum_m x[m,f] * aT[m,n]:  lhsT = x (as stored), rhs = aT.
#   - out[n,d] = sum_f t^T[f,n] * W[f,d]: lhsT = t^T, rhs = W (as stored),
#     which lands out in [n,d] layout for a direct DMA store.
#   - Matmuls run as float32r (full-rate fp32 mode; fp32 proper is 4x slower).
#     walrus requires f32r matmul operands to be produced by instructions that
#     round to f32r, so every operand tile is written by a DVE/ACT copy with
#     float32r output dtype (DMA-fed x/W get a one-time rounding copy).
#   - mask[n] = any(x[n,:] != 0), computed as sum(|x[n,:]|) > 0, and applied
#     fused into the ReLU: relu(mask * t) == mask * relu(t) since mask >= 0.

import numpy as np

B, N, F, D = 8, 2048, 512, 512
P = 128
NT = N // P        # 16 row-tiles of n (and of m, since a is square)
FT = F // P        # 4 tiles of f
NCHUNK = 512       # n is processed in chunks of 512 columns
NJ = N // NCHUNK   # 4
NSUB = NCHUNK // P # 4

_CACHE = {}


def _build_nc():
    from contextlib import ExitStack

    from concourse import bacc, mybir, tile
    from concourse.masks import make_identity

    f32 = mybir.dt.float32
    f32r = mybir.dt.float32r
    AF = mybir.ActivationFunctionType

    nc = bacc.Bacc(None)
    a_d = nc.dram_tensor("a", [N, N], f32, kind="ExternalInput")
    x_d = nc.dram_tensor("x", [N, F], f32, kind="ExternalInput")
    w_d = nc.dram_tensor("kernel", [F, D], f32, kind="ExternalInput")
    o_d = nc.dram_tensor("out", [N, D], f32, kind="ExternalOutput")

    with tile.TileContext(nc) as tc, ExitStack() as ctx:
        const = ctx.enter_context(tc.tile_pool(name="const", bufs=1))
        xp = ctx.enter_context(tc.tile_pool(name="xp", bufs=1))
        wp = ctx.enter_context(tc.tile_pool(name="wp", bufs=1))
        a_pool = ctx.enter_context(tc.tile_pool(name="a_pool", bufs=4))
        atp = ctx.enter_context(tc.tile_pool(name="atp", bufs=2))
        ttp = ctx.enter_context(tc.tile_pool(name="ttp", bufs=2))
        outp = ctx.enter_context(tc.tile_pool(name="outp", bufs=3))
        scr = ctx.enter_context(tc.tile_pool(name="scr", bufs=2))
        ps_tp = ctx.enter_context(tc.tile_pool(name="ps_tp", bufs=2, space="PSUM"))
        ps_mm = ctx.enter_context(tc.tile_pool(name="ps_mm", bufs=4, space="PSUM"))
        ps_o = ctx.enter_context(tc.tile_pool(name="ps_o", bufs=2, space="PSUM"))

        ident = const.tile([P, P], f32)
        make_identity(nc, ident[:])

        # x: DMA fp32 chunks into scratch, round to f32r resident tile
        # [p, 16, 512] (m on partitions). Same for W into [p, 4, 512].
        x_r = xp.tile([P, NT, F], f32r)
        for c in range(4):
            xl = scr.tile([P, NT // 4, F], f32, tag="load_scr")
            nc.sync.dma_start(
                xl[:],
                x_d[c * (N // 4) : (c + 1) * (N // 4), :].rearrange(
                    "(o p) f -> p o f", p=P
                ),
            )
            nc.vector.tensor_copy(x_r[:, c * 4 : (c + 1) * 4], xl[:])

        w_r = wp.tile([P, FT, D], f32r)
        wl = scr.tile([P, FT, D], f32, tag="load_scr")
        nc.sync.dma_start(wl[:], w_d[:].rearrange("(o p) d -> p o d", p=P))
        nc.vector.tensor_copy(w_r[:], wl[:])

        # mask[n] = (sum_f |x[n,f]|) > 0, one column per n-row-tile.
        # (|round_f32r(v)| > 0 iff |v| > 0: mantissa truncation keeps exponent.)
        sumabs = const.tile([P, NT], f32)
        mask_sb = const.tile([P, NT], f32)
        for ni in range(NT):
            abs_scr = scr.tile([P, F], f32, tag="abs_scr")
            nc.scalar.activation(
                abs_scr[:], x_r[:, ni], AF.Abs, accum_out=sumabs[:, ni : ni + 1]
            )
        nc.vector.tensor_scalar(
            mask_sb[:], sumabs[:], 0.0, None, mybir.AluOpType.is_gt
        )

        for nj in range(NJ):
            # Load the 4 n-strips of a for this chunk: a[nj*512:(nj+1)*512, :]
            a_strips = []
            for j in range(NSUB):
                ast = a_pool.tile([P, N], f32, tag="a_strip")
                ni = nj * NSUB + j
                nc.sync.dma_start(ast[:], a_d[ni * P : (ni + 1) * P, :])
                a_strips.append(ast)

            # Transpose to aT[m, n-chunk]: for each m-tile, 4 PE transposes into
            # one PSUM bank, then a single 512-wide rounding copyback
            # (alternating DVE/ACT to balance the two engines).
            at_sb = atp.tile([P, NT, NCHUNK], f32r, tag="at")
            for mi in range(NT):
                ps = ps_tp.tile([P, NCHUNK], f32, tag="pst")
                for j in range(NSUB):
                    nc.tensor.transpose(
                        ps[:, j * P : (j + 1) * P],
                        a_strips[j][:, mi * P : (mi + 1) * P],
                        ident[:],
                    )
                if mi % 2 == 0:
                    nc.vector.tensor_copy(at_sb[:, mi], ps[:])
                else:
                    nc.scalar.copy(at_sb[:, mi], ps[:])

            # t^T chunk [512 f, 512 n]: accumulate over the 16 m-tiles
            tt_sb = ttp.tile([P, FT, NCHUNK], f32r, tag="tt")
            for fi in range(FT):
                pt = ps_mm.tile([P, NCHUNK], f32, tag="psm")
                for mi in range(NT):
                    nc.tensor.matmul(
                        pt[:],
                        lhsT=x_r[:, mi, fi * P : (fi + 1) * P],
                        rhs=at_sb[:, mi],
                        start=(mi == 0),
                        stop=(mi == NT - 1),
                    )
                if fi % 2 == 0:
                    nc.vector.tensor_copy(tt_sb[:, fi], pt[:])
                else:
                    nc.scalar.copy(tt_sb[:, fi], pt[:])

            # out rows for this chunk: accumulate over the 4 f-tiles, then
            # fused relu+mask on ACT, then store.
            for ns in range(NSUB):
                po = ps_o.tile([P, D], f32, tag="pso")
                for fi in range(FT):
                    nc.tensor.matmul(
                        po[:],
                        lhsT=tt_sb[:, fi, ns * P : (ns + 1) * P],
                        rhs=w_r[:, fi],
                        start=(fi == 0),
                        stop=(fi == FT - 1),
                    )
                ni = nj * NSUB + ns
                ob = outp.tile([P, D], f32, tag="ob")
                nc.scalar.activation(
                    ob[:], po[:], AF.Relu, scale=mask_sb[:, ni : ni + 1]
                )
                nc.sync.dma_start(o_d[ni * P : (ni + 1) * P, :], ob[:])

    nc.compile()
    return nc


def get_nc():
    if "nc" not in _CACHE:
        _CACHE["nc"] = _build_nc()
    return _CACHE["nc"]


def kernel(**inputs) -> np.ndarray:
    from concourse.bass_utils import run_bass_kernel_spmd

    x = np.ascontiguousarray(np.asarray(inputs["x"], dtype=np.float32))
    a = np.ascontiguousarray(np.asarray(inputs["a"], dtype=np.float32))
    w = np.ascontiguousarray(np.asarray(inputs["kernel"], dtype=np.float32))
    assert x.shape == (B, N, F) and a.shape == (B, N, N) and w.shape == (F, D)

    nc = get_nc()
    in_maps = [{"a": a[b], "x": x[b], "kernel": w} for b in range(B)]
    res = run_bass_kernel_spmd(nc, in_maps, core_ids=list(range(B)))
    return np.stack([res.results[b]["out"] for b in range(B)], axis=0)


# revision 4
# speedup vs baseline: 1.0420x; 1.0420x over previous
# GCN layer kernel for Trainium2: out[b] = relu((a[b] @ x[b]) @ W) * mask[b]
#
# Sharding: data-parallel over the batch (graph) dim. B=8 graphs, 8 cores,
# one graph per core; W replicated. Inputs are the FULL tensors; shards are
# sliced host-side and the per-core outputs stacked back together.
#
# Per-core dataflow (a: [2048,2048], x: [2048,512], W: [512,512]):
#   - a must be contracted over its column index; TensorE contracts over the
#     partition (row) index of both operands, so a is transposed on-chip via
#     PE transpose (fp32 has no DMA-transpose path), 128x128 tiles.
#   - t^T[f,n] = sum_m x[m,f] * aT[m,n]:  lhsT = x (as stored), rhs = aT.
#   - out[n,d] = sum_f t^T[f,n] * W[f,d]: lhsT = t^T, rhs = W (as stored),
#     which lands out in [n,d] layout for a direct DMA store.
#   - Matmuls run as float32r (full-rate fp32 mode; fp32 proper is 4x slower).
#     walrus requires f32r matmul operands to be produced by instructions that
#     round to f32r, so every operand tile is written by a DVE/ACT copy with
#     float32r output dtype (DMA-fed x/W get a one-time rounding copy).
#   - mask[n] = any(x[n,:] != 0), computed as sum(|x[n,:]|) > 0, and applied
#     fused into the ReLU: relu(mask * t) == mask * relu(t) since mask >= 0.
#
# Schedule notes (from NTFF traces):
#   - A dozen fp32 identity matmuls run first, overlapping the initial DMA
#     wait, so the PE HAM clock-gate reaches K=8/8 (2.4 GHz) before real work.
#   - DMA order is x-chunk0, a-strips(nj=0), x-chunks 1-3, W: the PE ramps
#     with the DMA instead of idling behind a bulk x/W load.
#   - Transposes are grouped per a-strip (j-outer) so they start as soon as
#     strip 0 lands; the PSUM->SBUF copyback is a strided [128,4,128] copy.

import numpy as np

B, N, F, D = 8, 2048, 512, 512
P = 128
NT = N // P        # 16 row-tiles of n (and of m, since a is square)
FT = F // P        # 4 tiles of f
NCHUNK = 512       # n is processed in chunks of 512 columns
NJ = N // NCHUNK   # 4
NSUB = NCHUNK // P # 4

_CACHE = {}


def _build_nc():
    from contextlib import ExitStack

    from concourse import bacc, mybir, tile
    from concourse.masks import make_identity

    f32 = mybir.dt.float32
    f32r = mybir.dt.float32r
    AF = mybir.ActivationFunctionType

    nc = bacc.Bacc(None)
    a_d = nc.dram_tensor("a", [N, N], f32, kind="ExternalInput")
    x_d = nc.dram_tensor("x", [N, F], f32, kind="ExternalInput")
    w_d = nc.dram_tensor("kernel", [F, D], f32, kind="ExternalInput")
    o_d = nc.dram_tensor("out", [N, D], f32, kind="ExternalOutput")

    with tile.TileContext(nc) as tc, ExitStack() as ctx:
        const = ctx.enter_context(tc.tile_pool(name="const", bufs=1))
        xp = ctx.enter_context(tc.tile_pool(name="xp", bufs=1))
        wp = ctx.enter_context(tc.tile_pool(name="wp", bufs=1))
        a_pool = ctx.enter_context(tc.tile_pool(name="a_pool", bufs=5))
        atp = ctx.enter_context(tc.tile_pool(name="atp", bufs=2))
        ttp = ctx.enter_context(tc.tile_pool(name="ttp", bufs=2))
        outp = ctx.enter_context(tc.tile_pool(name="outp", bufs=3))
        scr = ctx.enter_context(tc.tile_pool(name="scr", bufs=2))
        ps_tp = ctx.enter_context(tc.tile_pool(name="ps_tp", bufs=2, space="PSUM"))
        ps_mm = ctx.enter_context(tc.tile_pool(name="ps_mm", bufs=4, space="PSUM"))
        ps_o = ctx.enter_context(tc.tile_pool(name="ps_o", bufs=2, space="PSUM"))

        ident = const.tile([P, P], f32)
        make_identity(nc, ident[:])

        # HAM warm-up: fp32 matmuls (counted as PE activity) spanning >3.4us
        # at the cold clock, so the K=8/8 un-throttle lands before real work.
        for wu in range(12):
            pw = ps_tp.tile([P, P], f32, tag="pst")
            nc.tensor.matmul(pw[:], lhsT=ident[:], rhs=ident[:], start=True, stop=True)

        # x: DMA fp32 column-chunks into scratch, round to f32r resident tile
        # [p, 16, 512] (m on partitions). Chunk 0 is issued before the a-strips
        # so mm1's first accumulation has its lhsT when the transposes finish.
        x_r = xp.tile([P, NT, F], f32r)
        xl0 = scr.tile([P, NT, P], f32, tag="load_scr")
        nc.sync.dma_start(
            xl0[:], x_d[:, 0:P].rearrange("(o p) f -> p o f", p=P)
        )
        nc.vector.tensor_copy(x_r[:, :, 0:P], xl0[:])

        # a-strips for nj=0 (the loop below skips its DMA for nj==0)
        first_strips = []
        for j in range(NSUB):
            ast = a_pool.tile([P, N], f32, tag="a_strip")
            nc.sync.dma_start(ast[:], a_d[j * P : (j + 1) * P, :])
            first_strips.append(ast)

        for c in range(1, 4):
            xl = scr.tile([P, NT, P], f32, tag="load_scr")
            nc.sync.dma_start(
                xl[:], x_d[:, c * P : (c + 1) * P].rearrange("(o p) f -> p o f", p=P)
            )
            nc.vector.tensor_copy(x_r[:, :, c * P : (c + 1) * P], xl[:])

        w_r = wp.tile([P, FT, D], f32r)
        wl = scr.tile([P, FT, D], f32, tag="load_scr")
        nc.sync.dma_start(wl[:], w_d[:].rearrange("(o p) d -> p o d", p=P))
        nc.vector.tensor_copy(w_r[:], wl[:])

        # mask[n] = (sum_f |x[n,f]|) > 0, one column per n-row-tile.
        # (|round_f32r(v)| > 0 iff |v| > 0: mantissa truncation keeps exponent.)
        sumabs = const.tile([P, NT], f32)
        mask_sb = const.tile([P, NT], f32)
        for ni in range(NT):
            abs_scr = scr.tile([P, F], f32, tag="abs_scr")
            nc.scalar.activation(
                abs_scr[:], x_r[:, ni], AF.Abs, accum_out=sumabs[:, ni : ni + 1]
            )
        nc.vector.tensor_scalar(
            mask_sb[:], sumabs[:], 0.0, None, mybir.AluOpType.is_gt
        )

        cb = 0  # copyback counter for DVE/ACT alternation
        for nj in range(NJ):
            # Load the 4 n-strips of a for this chunk: a[nj*512:(nj+1)*512, :]
            if nj == 0:
                a_strips = first_strips
            else:
                a_strips = []
                for j in range(NSUB):
                    ast = a_pool.tile([P, N], f32, tag="a_strip")
                    ni = nj * NSUB + j
                    nc.sync.dma_start(ast[:], a_d[ni * P : (ni + 1) * P, :])
                    a_strips.append(ast)

            # Transpose to aT[m, n-chunk], strip by strip (j-outer) so work
            # starts as soon as each strip lands. Four m-tiles share one PSUM
            # bank; one strided rounding copyback per quad, alternating
            # DVE/ACT to balance the two engines. (Transposes stay fp32: the
            # walrus f32r verifier requires rounded producers, and rounding
            # the DMA-fed a-strips would cost a full extra DVE pass; the
            # copyback CAST performs the f32r rounding instead.)
            at_sb = atp.tile([P, NT, NCHUNK], f32r, tag="at")
            for j in range(NSUB):
                src = a_strips[j][:]
                for q in range(NT // 4):
                    ps = ps_tp.tile([P, NCHUNK], f32, tag="pst")
                    for k in range(4):
                        mi = q * 4 + k
                        nc.tensor.transpose(
                            ps[:, k * P : (k + 1) * P],
                            src[:, mi * P : (mi + 1) * P],
                            ident[:],
                        )
                    dst = at_sb[:, q * 4 : (q + 1) * 4, j * P : (j + 1) * P]
                    src_q = ps[:].rearrange("p (q f) -> p q f", q=4)
                    if cb % 2 == 0:
                        nc.vector.tensor_copy(dst, src_q)
                    else:
                        nc.scalar.copy(dst, src_q)
                    cb += 1

            # t^T chunk [512 f, 512 n]: accumulate over the 16 m-tiles
            tt_sb = ttp.tile([P, FT, NCHUNK], f32r, tag="tt")
            for fi in range(FT):
                pt = ps_mm.tile([P, NCHUNK], f32, tag="psm")
                for mi in range(NT):
                    nc.tensor.matmul(
                        pt[:],
                        lhsT=x_r[:, mi, fi * P : (fi + 1) * P],
                        rhs=at_sb[:, mi],
                        start=(mi == 0),
                        stop=(mi == NT - 1),
                    )
                if fi % 2 == 0:
                    nc.vector.tensor_copy(tt_sb[:, fi], pt[:])
                else:
                    nc.scalar.copy(tt_sb[:, fi], pt[:])

            # out rows for this chunk: accumulate over the 4 f-tiles, then
            # fused relu+mask on ACT, then store.
            for ns in range(NSUB):
                po = ps_o.tile([P, D], f32, tag="pso")
                for fi in range(FT):
                    nc.tensor.matmul(
                        po[:],
                        lhsT=tt_sb[:, fi, ns * P : (ns + 1) * P],
                        rhs=w_r[:, fi],
                        start=(fi == 0),
                        stop=(fi == FT - 1),
                    )
                ni = nj * NSUB + ns
                ob = outp.tile([P, D], f32, tag="ob")
                nc.scalar.activation(
                    ob[:], po[:], AF.Relu, scale=mask_sb[:, ni : ni + 1]
                )
                nc.sync.dma_start(o_d[ni * P : (ni + 1) * P, :], ob[:])

    nc.compile()
    return nc


def get_nc():
    if "nc" not in _CACHE:
        _CACHE["nc"] = _build_nc()
    return _CACHE["nc"]


def kernel(**inputs) -> np.ndarray:
    from concourse.bass_utils import run_bass_kernel_spmd

    x = np.ascontiguousarray(np.asarray(inputs["x"], dtype=np.float32))
    a = np.ascontiguousarray(np.asarray(inputs["a"], dtype=np.float32))
    w = np.ascontiguousarray(np.asarray(inputs["kernel"], dtype=np.float32))
    assert x.shape == (B, N, F) and a.shape == (B, N, N) and w.shape == (F, D)

    nc = get_nc()
    in_maps = [{"a": a[b], "x": x[b], "kernel": w} for b in range(B)]
    res = run_bass_kernel_spmd(nc, in_maps, core_ids=list(range(B)))
    return np.stack([res.results[b]["out"] for b in range(B)], axis=0)
